# revision 1
# baseline (speedup 1.0000x reference)
"""Chamfer loss kernel for 8x Trainium2 NeuronCores.

Problem: pred [4, 8192, 32] f32, target [4, 8192, 32] f32 ->
scalar = mean_n min_m ||p_n - t_m|| + mean_m min_n ||p_n - t_m||
(per batch, averaged over batch and points).

Sharding: batch b (4) x row-half h (2) -> 8 cores. Core c = 2*b + h
handles pred rows [h*4096, (h+1)*4096) of batch b against the full
target of batch b.

Device kernel (per core): an augmented K=34 matmul produces the full
squared-distance tile d2[n, m] directly in PSUM:
    lhsT rows 0-31 = -2 * pred^T, row 32 = |p_n|^2, row 33 = 1
    rhs  rows 0-31 = target^T,    row 32 = 1,       row 33 = |t_m|^2
Then the Vector engine does a free-axis min-reduce per tile (row
minima) and an elementwise min-accumulate across row tiles (column
minima per partition). Host combines: partition-min for columns,
min across the two cores of a batch, sqrt, and the means.
"""

import sys

sys.path.insert(0, "/opt/trn_rl_repo")

import numpy as np

B, N, M, D = 4, 8192, 8192, 32
N_LOC = N // 2          # rows per core
K_AUG = D + 2           # 34
NI = N_LOC // 128       # 32 row tiles
NJ = M // 512           # 16 column chunks

_compiled = None


def _build():
    import concourse.bacc as bacc
    import concourse.mybir as mybir
    import concourse.tile as tile

    nc = bacc.Bacc("TRN2", target_bir_lowering=False, debug=False, num_devices=8)
    pt_d = nc.dram_tensor("pt", [K_AUG, N_LOC], mybir.dt.float32, kind="ExternalInput")
    tt_d = nc.dram_tensor("tt", [K_AUG, M], mybir.dt.float32, kind="ExternalInput")
    row_d = nc.dram_tensor("rowmin", [128, NI], mybir.dt.float32, kind="ExternalOutput")
    col_d = nc.dram_tensor(
        "colmin", [128, NJ, 512], mybir.dt.float32, kind="ExternalOutput"
    )

    f32 = mybir.dt.float32
    AX = mybir.AxisListType
    OP = mybir.AluOpType

    with tile.TileContext(nc) as tc:
        with (
            tc.tile_pool(name="const", bufs=1) as const,
            tc.tile_pool(name="psum", bufs=8, space="PSUM") as psum_pool,
        ):
            ptsb = const.tile([K_AUG, N_LOC], f32)
            ttsb = const.tile([K_AUG, M], f32)
            nc.sync.dma_start(out=ptsb[:], in_=pt_d.ap())
            nc.sync.dma_start(out=ttsb[:], in_=tt_d.ap())

            # rowbuf[p, i, j] = min over m-chunk j of d2 for local row 128*i+p
            rowbuf = const.tile([128, NI, NJ], f32)
            # colbuf[p, j, q] = min over row tiles i of d2[128i+p, 512j+q]
            colbuf = const.tile([128, NJ, 512], f32)

            for i in range(NI):
                lhsT = ptsb[:, i * 128 : (i + 1) * 128]
                for j in range(NJ):
                    ps = psum_pool.tile([128, 512], f32)
                    nc.tensor.matmul(
                        ps[:],
                        lhsT,
                        ttsb[:, j * 512 : (j + 1) * 512],
                        start=True,
                        stop=True,
                    )
                    nc.vector.tensor_reduce(
                        rowbuf[:, i, j : j + 1], ps[:], axis=AX.X, op=OP.min
                    )
                    if i == 0:
                        nc.scalar.copy(colbuf[:, j, :], ps[:])
                    else:
                        nc.vector.tensor_tensor(
                            colbuf[:, j, :], ps[:], colbuf[:, j, :], op=OP.min
                        )

            rowout = const.tile([128, NI], f32)
            nc.vector.tensor_reduce(rowout[:], rowbuf[:], axis=AX.X, op=OP.min)
            nc.sync.dma_start(out=row_d.ap(), in_=rowout[:])
            nc.sync.dma_start(out=col_d.ap(), in_=colbuf[:])

    nc.compile()
    return nc


def _get_compiled():
    global _compiled
    if _compiled is None:
        _compiled = _build()
    return _compiled


def _make_core_inputs(pred, target):
    """Per-core augmented, transposed operands."""
    ins = []
    for c in range(8):
        b, h = c // 2, c % 2
        pl = pred[b, h * N_LOC : (h + 1) * N_LOC]  # [N_LOC, 32]
        tg = target[b]  # [M, 32]
        pt = np.empty((K_AUG, N_LOC), dtype=np.float32)
        pt[:D] = -2.0 * pl.T
        pt[D] = np.sum(pl * pl, axis=1)
        pt[D + 1] = 1.0
        tt = np.empty((K_AUG, M), dtype=np.float32)
        tt[:D] = tg.T
        tt[D] = 1.0
        tt[D + 1] = np.sum(tg * tg, axis=1)
        ins.append({"pt": np.ascontiguousarray(pt), "tt": np.ascontiguousarray(tt)})
    return ins


def _finish(results):
    """Host tail: combine per-core partial minima into the scalar loss."""
    row_sum = 0.0
    col_sum = 0.0
    for b in range(B):
        col_d2 = None
        for h in range(2):
            r = results[2 * b + h]
            # rows: rowmin[p, i] is row n = 128*i + p
            rm = r["rowmin"]  # [128, NI]
            row_sum += np.sum(np.sqrt(np.maximum(rm, 0.0)))
            # cols: min over partitions -> [NJ, 512] -> [M]
            cm = r["colmin"].min(axis=0).reshape(M)
            col_d2 = cm if col_d2 is None else np.minimum(col_d2, cm)
        col_sum += np.sum(np.sqrt(np.maximum(col_d2, 0.0)))
    total = row_sum / (B * N) + col_sum / (B * M)
    return np.float32(total)


def kernel(pred, target, **run_kwargs):
    from concourse.bass_utils import run_bass_kernel_spmd

    pred = np.asarray(pred, dtype=np.float32)
    target = np.asarray(target, dtype=np.float32)
    nc = _get_compiled()
    ins = _make_core_inputs(pred, target)
    res = run_bass_kernel_spmd(nc, ins, list(range(8)), **run_kwargs)
    out = _finish(res.results)
    if run_kwargs:
        return out, res
    return out


# revision 3
# speedup vs baseline: 2.4544x; 2.4544x over previous
"""Chamfer loss kernel for 8x Trainium2 NeuronCores.

Problem: pred [4, 8192, 32] f32, target [4, 8192, 32] f32 ->
scalar = mean_n min_m ||p_n - t_m|| + mean_m min_n ||p_n - t_m||
(per batch, averaged over batch and points).

Sharding: batch b (4) x row-half h (2) -> 8 cores. Core c = 2*b + h
handles pred rows [h*4096, (h+1)*4096) of batch b against the full
target of batch b.

Device kernel (per core): an augmented K=34 bf16 matmul produces the
full squared-distance tile d2[n, m] directly in PSUM (fp32 accum):
    lhsT rows 0-31 = -2 * pred^T, row 32 = |p_n|^2, row 33 = 1
    rhs  rows 0-31 = target^T,    row 32 = 1,       row 33 = |t_m|^2
The Scalar engine casts PSUM spans to bf16 in SBUF; the Vector engine
then computes row minima via a pair-min tree (bf16 2x mode) + reduce,
and column minima via an elementwise min-accumulate across row tiles.
Host combines: partition-min for columns, min across the two cores of
a batch, sqrt, means.  bf16 rounding of the distance candidates gives
~4e-5 relative error on the final loss (validated against fp32).
"""

import sys

sys.path.insert(0, "/opt/trn_rl_repo")

import ml_dtypes
import numpy as np

B, N, M, D = 4, 8192, 8192, 32
N_LOC = N // 2          # rows per core
K_AUG = D + 2           # 34
NI = N_LOC // 128       # 32 row tiles
SPAN = 2048             # m-elements per DVE span (4 PSUM banks)
NJJ = M // SPAN         # 4 column spans

_compiled = None


def _build():
    import concourse.bacc as bacc
    import concourse.mybir as mybir
    import concourse.tile as tile

    nc = bacc.Bacc("TRN2", target_bir_lowering=False, debug=False, num_devices=8)
    f32 = mybir.dt.float32
    bf16 = mybir.dt.bfloat16
    AX = mybir.AxisListType
    OP = mybir.AluOpType

    pt_d = nc.dram_tensor("pt", [K_AUG, N_LOC], bf16, kind="ExternalInput")
    tt_d = nc.dram_tensor("tt", [K_AUG, M], bf16, kind="ExternalInput")
    row_d = nc.dram_tensor("rowmin", [128, NI], f32, kind="ExternalOutput")
    col_d = nc.dram_tensor("colmin", [128, NJJ, SPAN], bf16, kind="ExternalOutput")

    with tile.TileContext(nc) as tc:
        with (
            tc.tile_pool(name="const", bufs=1) as const,
            tc.tile_pool(name="psum", bufs=2, space="PSUM") as psum_pool,
            tc.tile_pool(name="sbbf", bufs=4) as sbbf_pool,
            tc.tile_pool(name="tree", bufs=4) as tree_pool,
        ):
            ptsb = const.tile([K_AUG, N_LOC], bf16)
            ttsb = const.tile([K_AUG, M], bf16)
            nc.sync.dma_start(out=ptsb[:], in_=pt_d.ap())
            nc.sync.dma_start(out=ttsb[:], in_=tt_d.ap())

            # rowbuf[p, i, jj] = min over m-span jj of d2 for local row 128*i+p
            rowbuf = const.tile([128, NI, NJJ], f32)
            # colbuf[p, jj, q] = min over row tiles i of d2[128i+p, SPAN*jj+q]
            colbuf = const.tile([128, NJJ, SPAN], bf16)
            nc.vector.memset(colbuf[:], float("inf"))

            for i in range(NI):
                lhsT = ptsb[:, i * 128 : (i + 1) * 128]
                for jj in range(NJJ):
                    ps = psum_pool.tile([128, SPAN], f32)
                    for h in range(SPAN // 512):
                        nc.tensor.matmul(
                            ps[:, h * 512 : (h + 1) * 512],
                            lhsT,
                            ttsb[:, jj * SPAN + h * 512 : jj * SPAN + (h + 1) * 512],
                            start=True,
                            stop=True,
                        )
                    sb = sbbf_pool.tile([128, SPAN], bf16)
                    nc.scalar.copy(sb[:], ps[:])
                    # row minima: bf16 pair-min tree then reduce
                    u = tree_pool.tile([128, SPAN // 2], bf16, tag="u")
                    nc.vector.tensor_tensor(
                        u[:], sb[:, : SPAN // 2], sb[:, SPAN // 2 :], op=OP.min
                    )
                    v = tree_pool.tile([128, SPAN // 4], bf16, tag="v")
                    nc.vector.tensor_tensor(
                        v[:], u[:, : SPAN // 4], u[:, SPAN // 4 :], op=OP.min
                    )
                    nc.vector.tensor_reduce(
                        rowbuf[:, i, jj : jj + 1], v[:], axis=AX.X, op=OP.min
                    )
                    # column minima accumulate
                    nc.vector.tensor_tensor(
                        colbuf[:, jj, :], sb[:], colbuf[:, jj, :], op=OP.min
                    )

            rowout = const.tile([128, NI], f32)
            nc.vector.tensor_reduce(rowout[:], rowbuf[:], axis=AX.X, op=OP.min)
            nc.sync.dma_start(out=row_d.ap(), in_=rowout[:])
            nc.sync.dma_start(out=col_d.ap(), in_=colbuf[:])

    nc.compile()
    return nc


def _get_compiled():
    global _compiled
    if _compiled is None:
        _compiled = _build()
    return _compiled


def _make_core_inputs(pred, target):
    """Per-core augmented, transposed bf16 operands."""
    ins = []
    for c in range(8):
        b, h = c // 2, c % 2
        pl = pred[b, h * N_LOC : (h + 1) * N_LOC]  # [N_LOC, 32]
        tg = target[b]  # [M, 32]
        pt = np.empty((K_AUG, N_LOC), dtype=np.float32)
        pt[:D] = -2.0 * pl.T
        pt[D] = np.sum(pl * pl, axis=1)
        pt[D + 1] = 1.0
        tt = np.empty((K_AUG, M), dtype=np.float32)
        tt[:D] = tg.T
        tt[D] = 1.0
        tt[D + 1] = np.sum(tg * tg, axis=1)
        ins.append(
            {
                "pt": np.ascontiguousarray(pt.astype(ml_dtypes.bfloat16)),
                "tt": np.ascontiguousarray(tt.astype(ml_dtypes.bfloat16)),
            }
        )
    return ins


def _finish(results):
    """Host tail: combine per-core partial minima into the scalar loss."""
    row_sum = 0.0
    col_sum = 0.0
    for b in range(B):
        col_d2 = None
        for h in range(2):
            r = results[2 * b + h]
            rm = np.asarray(r["rowmin"], dtype=np.float64)  # [128, NI]
            row_sum += np.sum(np.sqrt(np.maximum(rm, 0.0)))
            cm = np.asarray(r["colmin"], dtype=np.float64).min(axis=0).reshape(M)
            col_d2 = cm if col_d2 is None else np.minimum(col_d2, cm)
        col_sum += np.sum(np.sqrt(np.maximum(col_d2, 0.0)))
    total = row_sum / (B * N) + col_sum / (B * M)
    return np.float32(total)


def kernel(pred, target, **run_kwargs):
    from concourse.bass_utils import run_bass_kernel_spmd

    pred = np.asarray(pred, dtype=np.float32)
    target = np.asarray(target, dtype=np.float32)
    nc = _get_compiled()
    ins = _make_core_inputs(pred, target)
    res = run_bass_kernel_spmd(nc, ins, list(range(8)), **run_kwargs)
    out = _finish(res.results)
    if run_kwargs:
        return out, res
    return out


# revision 4
# speedup vs baseline: 2.6696x; 1.0877x over previous
"""Chamfer loss kernel for 8x Trainium2 NeuronCores.

Problem: pred [4, 8192, 32] f32, target [4, 8192, 32] f32 ->
scalar = mean_n min_m ||p_n - t_m|| + mean_m min_n ||p_n - t_m||
(per batch, averaged over batch and points).

Sharding: batch b (4) x row-half h (2) -> 8 cores. Core c = 2*b + h
handles pred rows [h*4096, (h+1)*4096) of batch b against the full
target of batch b.

Device kernel (per core): an augmented K=34 fp16 matmul produces the
full squared-distance tile d2[n, m] directly in PSUM (fp32 accum):
    lhsT rows 0-31 = -2 * pred^T, row 32 = |p_n|^2, row 33 = 1
    rhs  rows 0-31 = target^T,    row 32 = 1,       row 33 = |t_m|^2
The Scalar engine casts PSUM spans to fp16 in SBUF; the Vector engine
computes row minima via a fp16 pair-min tree (2x mode) + per-row-tile
accumulator, and column minima via an elementwise min-accumulate over
row tiles. Host combines: partition-min for columns, min across the
two cores of a batch, sqrt, means. fp16 rounding of the distance
candidates costs ~1e-5 relative error on the final loss.
"""

import sys

sys.path.insert(0, "/opt/trn_rl_repo")

import ml_dtypes
import numpy as np

B, N, M, D = 4, 8192, 8192, 32
N_LOC = N // 2          # rows per core
K_AUG = D + 2           # 34
NI = N_LOC // 128       # 32 row tiles
SPAN = 2048             # m-elements per DVE span (4 PSUM banks)
NJJ = M // SPAN         # 4 column spans

_compiled = None


def _build():
    import concourse.bacc as bacc
    import concourse.mybir as mybir
    import concourse.tile as tile

    nc = bacc.Bacc("TRN2", target_bir_lowering=False, debug=False, num_devices=8)
    f32 = mybir.dt.float32
    f16 = mybir.dt.float16
    AX = mybir.AxisListType
    OP = mybir.AluOpType

    pt_d = nc.dram_tensor("pt", [K_AUG, N_LOC], f16, kind="ExternalInput")
    tt_d = nc.dram_tensor("tt", [K_AUG, M], f16, kind="ExternalInput")
    row_d = nc.dram_tensor("rowmin", [128, NI], f32, kind="ExternalOutput")
    col_d = nc.dram_tensor("colmin", [128, NJJ, SPAN], f16, kind="ExternalOutput")

    with tile.TileContext(nc) as tc:
        with (
            tc.tile_pool(name="const", bufs=1) as const,
            tc.tile_pool(name="psum", bufs=2, space="PSUM") as psum_pool,
            tc.tile_pool(name="sbbf", bufs=4) as sbbf_pool,
            tc.tile_pool(name="tree", bufs=4) as tree_pool,
        ):
            ptsb = const.tile([K_AUG, N_LOC], f16)
            ttsb = const.tile([K_AUG, M], f16)
            nc.sync.dma_start(out=ptsb[:], in_=pt_d.ap())
            nc.sync.dma_start(out=ttsb[:], in_=tt_d.ap())

            # rowacc[p, i, :] accumulates 512-wide row-min candidates for
            # row tile i; one final reduce per i produces rowmin.
            rowacc = const.tile([128, NI, 512], f16)
            # colbuf[p, jj, q] = min over row tiles i of d2[128i+p, SPAN*jj+q]
            colbuf = const.tile([128, NJJ, SPAN], f16)

            for i in range(NI):
                lhsT = ptsb[:, i * 128 : (i + 1) * 128]
                for jj in range(NJJ):
                    ps = psum_pool.tile([128, SPAN], f32)
                    for h in range(SPAN // 512):
                        nc.tensor.matmul(
                            ps[:, h * 512 : (h + 1) * 512],
                            lhsT,
                            ttsb[:, jj * SPAN + h * 512 : jj * SPAN + (h + 1) * 512],
                            start=True,
                            stop=True,
                        )
                    sb = sbbf_pool.tile([128, SPAN], f16)
                    nc.scalar.copy(sb[:], ps[:])
                    # row minima: fp16 pair-min tree into per-i accumulator
                    u = tree_pool.tile([128, SPAN // 2], f16, tag="u")
                    nc.vector.tensor_tensor(
                        u[:], sb[:, : SPAN // 2], sb[:, SPAN // 2 :], op=OP.min
                    )
                    if jj == 0:
                        nc.vector.tensor_tensor(
                            rowacc[:, i, :], u[:, : SPAN // 4], u[:, SPAN // 4 :],
                            op=OP.min,
                        )
                    else:
                        v = tree_pool.tile([128, SPAN // 4], f16, tag="v")
                        nc.vector.tensor_tensor(
                            v[:], u[:, : SPAN // 4], u[:, SPAN // 4 :], op=OP.min
                        )
                        nc.vector.tensor_tensor(
                            rowacc[:, i, :], v[:], rowacc[:, i, :], op=OP.min
                        )
                    # column minima accumulate
                    if i == 0:
                        nc.vector.tensor_copy(colbuf[:, jj, :], sb[:])
                    else:
                        nc.vector.tensor_tensor(
                            colbuf[:, jj, :], sb[:], colbuf[:, jj, :], op=OP.min
                        )

            rowout = const.tile([128, NI], f32)
            for i in range(NI):
                nc.vector.tensor_reduce(
                    rowout[:, i : i + 1], rowacc[:, i, :], axis=AX.X, op=OP.min
                )
            nc.sync.dma_start(out=row_d.ap(), in_=rowout[:])
            nc.sync.dma_start(out=col_d.ap(), in_=colbuf[:])

    nc.compile()
    return nc


def _get_compiled():
    global _compiled
    if _compiled is None:
        _compiled = _build()
    return _compiled


def _make_core_inputs(pred, target):
    """Per-core augmented, transposed fp16 operands."""
    ins = []
    for c in range(8):
        b, h = c // 2, c % 2
        pl = pred[b, h * N_LOC : (h + 1) * N_LOC]  # [N_LOC, 32]
        tg = target[b]  # [M, 32]
        pt = np.empty((K_AUG, N_LOC), dtype=np.float32)
        pt[:D] = -2.0 * pl.T
        pt[D] = np.sum(pl * pl, axis=1)
        pt[D + 1] = 1.0
        tt = np.empty((K_AUG, M), dtype=np.float32)
        tt[:D] = tg.T
        tt[D] = 1.0
        tt[D + 1] = np.sum(tg * tg, axis=1)
        ins.append(
            {
                "pt": np.ascontiguousarray(pt.astype(np.float16)),
                "tt": np.ascontiguousarray(tt.astype(np.float16)),
            }
        )
    return ins


def _finish(results):
    """Host tail: combine per-core partial minima into the scalar loss."""
    row_sum = 0.0
    col_sum = 0.0
    for b in range(B):
        col_d2 = None
        for h in range(2):
            r = results[2 * b + h]
            rm = np.asarray(r["rowmin"], dtype=np.float64)  # [128, NI]
            row_sum += np.sum(np.sqrt(np.maximum(rm, 0.0)))
            cm = np.asarray(r["colmin"], dtype=np.float64).min(axis=0).reshape(M)
            col_d2 = cm if col_d2 is None else np.minimum(col_d2, cm)
        col_sum += np.sum(np.sqrt(np.maximum(col_d2, 0.0)))
    total = row_sum / (B * N) + col_sum / (B * M)
    return np.float32(total)


def kernel(pred, target, **run_kwargs):
    from concourse.bass_utils import run_bass_kernel_spmd

    pred = np.asarray(pred, dtype=np.float32)
    target = np.asarray(target, dtype=np.float32)
    nc = _get_compiled()
    ins = _make_core_inputs(pred, target)
    res = run_bass_kernel_spmd(nc, ins, list(range(8)), **run_kwargs)
    out = _finish(res.results)
    if run_kwargs:
        return out, res
    return out


# revision 6
# speedup vs baseline: 2.6698x; 1.0001x over previous
"""Chamfer loss kernel for 8x Trainium2 NeuronCores.

Problem: pred [4, 8192, 32] f32, target [4, 8192, 32] f32 ->
scalar = mean_n min_m ||p_n - t_m|| + mean_m min_n ||p_n - t_m||
(per batch, averaged over batch and points).

Sharding: batch b (4) x row-half h (2) -> 8 cores. Core c = 2*b + h
handles pred rows [h*4096, (h+1)*4096) of batch b against the full
target of batch b.

Device kernel (per core): an augmented K=34 fp16 matmul produces the
full squared-distance tile d2[n, m] directly in PSUM (fp32 accum):
    lhsT rows 0-31 = -2 * pred^T, row 32 = |p_n|^2, row 33 = 1
    rhs  rows 0-31 = target^T,    row 32 = 1,       row 33 = |t_m|^2
The Scalar engine casts PSUM spans to fp16 in SBUF; the Vector engine
computes row minima via a fp16 pair-min tree (2x mode) + per-row-tile
accumulator, and column minima via an elementwise min-accumulate over
row tiles. Host combines: partition-min for columns, min across the
two cores of a batch, sqrt, means. fp16 rounding of the distance
candidates costs ~1e-5 relative error on the final loss.
"""

import sys

sys.path.insert(0, "/opt/trn_rl_repo")

import ml_dtypes
import numpy as np

B, N, M, D = 4, 8192, 8192, 32
N_LOC = N // 2          # rows per core
K_AUG = D + 2           # 34
NI = N_LOC // 128       # 32 row tiles
SPAN = 2048             # m-elements per DVE span (4 PSUM banks)
NJJ = M // SPAN         # 4 column spans

_compiled = None


def _build():
    import concourse.bacc as bacc
    import concourse.mybir as mybir
    import concourse.tile as tile

    nc = bacc.Bacc("TRN2", target_bir_lowering=False, debug=False, num_devices=8)
    f32 = mybir.dt.float32
    f16 = mybir.dt.float16
    AX = mybir.AxisListType
    OP = mybir.AluOpType

    pt_d = nc.dram_tensor("pt", [K_AUG, N_LOC], f16, kind="ExternalInput")
    tt_d = nc.dram_tensor("tt", [K_AUG, M], f16, kind="ExternalInput")
    row_d = nc.dram_tensor("rowmin", [128, NI], f32, kind="ExternalOutput")
    col_d = nc.dram_tensor("colmin", [128, NJJ, SPAN], f16, kind="ExternalOutput")

    with tile.TileContext(nc) as tc:
        with (
            tc.tile_pool(name="const", bufs=1) as const,
            tc.tile_pool(name="psum", bufs=2, space="PSUM") as psum_pool,
            tc.tile_pool(name="sbbf", bufs=4) as sbbf_pool,
            tc.tile_pool(name="tree", bufs=4) as tree_pool,
        ):
            ptsb = const.tile([K_AUG, N_LOC], f16)
            ttsb = const.tile([K_AUG, M], f16)
            nc.sync.dma_start(out=ptsb[:], in_=pt_d.ap())
            nc.sync.dma_start(out=ttsb[:], in_=tt_d.ap())

            # rowacc[p, i, :] accumulates 1024-wide row-min candidates for
            # row tile i; one final reduce per i produces rowmin.
            rowacc = const.tile([128, NI, 1024], f16)
            # colbuf[p, jj, q] = min over row tiles i of d2[128i+p, SPAN*jj+q]
            colbuf = const.tile([128, NJJ, SPAN], f16)

            for i in range(NI):
                lhsT = ptsb[:, i * 128 : (i + 1) * 128]
                for jj in range(NJJ):
                    ps = psum_pool.tile([128, SPAN], f32)
                    for h in range(SPAN // 512):
                        nc.tensor.matmul(
                            ps[:, h * 512 : (h + 1) * 512],
                            lhsT,
                            ttsb[:, jj * SPAN + h * 512 : jj * SPAN + (h + 1) * 512],
                            start=True,
                            stop=True,
                        )
                    sb = sbbf_pool.tile([128, SPAN], f16)
                    nc.scalar.copy(sb[:], ps[:])
                    # row minima: fp16 pair-min into per-i accumulator
                    if jj == 0:
                        # first span writes the accumulator directly
                        nc.vector.tensor_tensor(
                            rowacc[:, i, :], sb[:, : SPAN // 2], sb[:, SPAN // 2 :],
                            op=OP.min,
                        )
                    else:
                        u = tree_pool.tile([128, SPAN // 2], f16, tag="u")
                        nc.vector.tensor_tensor(
                            u[:], sb[:, : SPAN // 2], sb[:, SPAN // 2 :], op=OP.min
                        )
                        nc.vector.tensor_tensor(
                            rowacc[:, i, :], u[:], rowacc[:, i, :], op=OP.min
                        )
                    # column minima accumulate
                    if i == 0:
                        nc.vector.tensor_copy(colbuf[:, jj, :], sb[:])
                    else:
                        nc.vector.tensor_tensor(
                            colbuf[:, jj, :], sb[:], colbuf[:, jj, :], op=OP.min
                        )

            rowout = const.tile([128, NI], f32)
            for i in range(NI):
                nc.vector.tensor_reduce(
                    rowout[:, i : i + 1], rowacc[:, i, :], axis=AX.X, op=OP.min
                )
            nc.sync.dma_start(out=row_d.ap(), in_=rowout[:])
            nc.sync.dma_start(out=col_d.ap(), in_=colbuf[:])

    nc.compile()
    return nc


def _get_compiled():
    global _compiled
    if _compiled is None:
        _compiled = _build()
    return _compiled


def _make_core_inputs(pred, target):
    """Per-core augmented, transposed fp16 operands."""
    ins = []
    for c in range(8):
        b, h = c // 2, c % 2
        pl = pred[b, h * N_LOC : (h + 1) * N_LOC]  # [N_LOC, 32]
        tg = target[b]  # [M, 32]
        pt = np.empty((K_AUG, N_LOC), dtype=np.float32)
        pt[:D] = -2.0 * pl.T
        pt[D] = np.sum(pl * pl, axis=1)
        pt[D + 1] = 1.0
        tt = np.empty((K_AUG, M), dtype=np.float32)
        tt[:D] = tg.T
        tt[D] = 1.0
        tt[D + 1] = np.sum(tg * tg, axis=1)
        ins.append(
            {
                "pt": np.ascontiguousarray(pt.astype(np.float16)),
                "tt": np.ascontiguousarray(tt.astype(np.float16)),
            }
        )
    return ins


def _finish(results):
    """Host tail: combine per-core partial minima into the scalar loss."""
    row_sum = 0.0
    col_sum = 0.0
    for b in range(B):
        col_d2 = None
        for h in range(2):
            r = results[2 * b + h]
            rm = np.asarray(r["rowmin"], dtype=np.float64)  # [128, NI]
            row_sum += np.sum(np.sqrt(np.maximum(rm, 0.0)))
            cm = np.asarray(r["colmin"], dtype=np.float64).min(axis=0).reshape(M)
            col_d2 = cm if col_d2 is None else np.minimum(col_d2, cm)
        col_sum += np.sum(np.sqrt(np.maximum(col_d2, 0.0)))
    total = row_sum / (B * N) + col_sum / (B * M)
    return np.float32(total)


def kernel(pred, target, **run_kwargs):
    from concourse.bass_utils import run_bass_kernel_spmd

    pred = np.asarray(pred, dtype=np.float32)
    target = np.asarray(target, dtype=np.float32)
    nc = _get_compiled()
    ins = _make_core_inputs(pred, target)
    res = run_bass_kernel_spmd(nc, ins, list(range(8)), **run_kwargs)
    out = _finish(res.results)
    if run_kwargs:
        return out, res
    return out


# revision 7
# speedup vs baseline: 2.9615x; 1.1093x over previous
"""Chamfer loss kernel for 8x Trainium2 NeuronCores.

Problem: pred [4, 8192, 32] f32, target [4, 8192, 32] f32 ->
scalar = mean_n min_m ||p_n - t_m|| + mean_m min_n ||p_n - t_m||
(per batch, averaged over batch and points).

Sharding: batch b (4) x row-half h (2) -> 8 cores. Core c = 2*b + h
handles pred rows [h*4096, (h+1)*4096) of batch b against the full
target of batch b.

Device kernel (per core): an augmented K=34 fp16 matmul produces the
full squared-distance tile d2[n, m] directly in PSUM (fp32 accum):
    lhsT rows 0-31 = -2 * pred^T, row 32 = |p_n|^2, row 33 = 1
    rhs  rows 0-31 = target^T,    row 32 = 1,       row 33 = |t_m|^2
The Scalar engine casts PSUM spans to fp16 in SBUF. The Vector engine
runs two fp16 pair-min tree levels (2x mode) for the row direction and
an elementwise min-accumulate over row tiles for the column direction.
512-wide row-min candidates are staged and DMA'd out (DMA engines are
otherwise idle); the host finishes both reductions (free-axis min for
rows, partition min for columns), combines the two cores of each
batch, applies sqrt and the means. fp16 rounding of the distance
candidates costs ~3e-6 relative error on the final loss.
"""

import sys

sys.path.insert(0, "/opt/trn_rl_repo")

import ml_dtypes
import numpy as np

B, N, M, D = 4, 8192, 8192, 32
N_LOC = N // 2          # rows per core
K_AUG = D + 2           # 34
NI = N_LOC // 128       # 32 row tiles
SPAN = 2048             # m-elements per DVE span (4 PSUM banks)
NJJ = M // SPAN         # 4 column spans
IGRP = 8                # row tiles per staging flush

_compiled = None


def _build():
    import concourse.bacc as bacc
    import concourse.mybir as mybir
    import concourse.tile as tile

    nc = bacc.Bacc("TRN2", target_bir_lowering=False, debug=False, num_devices=8)
    f32 = mybir.dt.float32
    f16 = mybir.dt.float16
    OP = mybir.AluOpType

    pt_d = nc.dram_tensor("pt", [K_AUG, N_LOC], f16, kind="ExternalInput")
    tt_d = nc.dram_tensor("tt", [K_AUG, M], f16, kind="ExternalInput")
    row_d = nc.dram_tensor(
        "rowcand", [128, NI, NJJ, 512], f16, kind="ExternalOutput"
    )
    col_d = nc.dram_tensor("colmin", [128, NJJ, SPAN], f16, kind="ExternalOutput")

    with tile.TileContext(nc) as tc:
        with (
            tc.tile_pool(name="const", bufs=1) as const,
            tc.tile_pool(name="psum", bufs=2, space="PSUM") as psum_pool,
            tc.tile_pool(name="sbbf", bufs=4) as sbbf_pool,
            tc.tile_pool(name="tree", bufs=4) as tree_pool,
            tc.tile_pool(name="stage", bufs=2) as stage_pool,
        ):
            ptsb = const.tile([K_AUG, N_LOC], f16)
            ttsb = const.tile([K_AUG, M], f16)
            nc.sync.dma_start(out=ptsb[:], in_=pt_d.ap())
            nc.sync.dma_start(out=ttsb[:], in_=tt_d.ap())

            # colbuf[p, jj, q] = min over row tiles i of d2[128i+p, SPAN*jj+q]
            colbuf = const.tile([128, NJJ, SPAN], f16)

            rowstage = None
            for i in range(NI):
                if i % IGRP == 0:
                    rowstage = stage_pool.tile([128, IGRP, NJJ, 512], f16)
                lhsT = ptsb[:, i * 128 : (i + 1) * 128]
                for jj in range(NJJ):
                    ps = psum_pool.tile([128, SPAN], f32)
                    for h in range(SPAN // 512):
                        nc.tensor.matmul(
                            ps[:, h * 512 : (h + 1) * 512],
                            lhsT,
                            ttsb[:, jj * SPAN + h * 512 : jj * SPAN + (h + 1) * 512],
                            start=True,
                            stop=True,
                        )
                    sb = sbbf_pool.tile([128, SPAN], f16)
                    nc.scalar.copy(sb[:], ps[:])
                    # row direction: two fp16 pair-min tree levels
                    u = tree_pool.tile([128, SPAN // 2], f16, tag="u")
                    nc.vector.tensor_tensor(
                        u[:], sb[:, : SPAN // 2], sb[:, SPAN // 2 :], op=OP.min
                    )
                    nc.vector.tensor_tensor(
                        rowstage[:, i % IGRP, jj, :],
                        u[:, : SPAN // 4],
                        u[:, SPAN // 4 :],
                        op=OP.min,
                    )
                    # column direction: min-accumulate over row tiles
                    if i == 0:
                        nc.vector.tensor_copy(colbuf[:, jj, :], sb[:])
                    else:
                        nc.vector.tensor_tensor(
                            colbuf[:, jj, :], sb[:], colbuf[:, jj, :], op=OP.min
                        )
                if i % IGRP == IGRP - 1:
                    g = i // IGRP
                    nc.sync.dma_start(
                        out=row_d.ap()[:, g * IGRP : (g + 1) * IGRP, :, :],
                        in_=rowstage[:],
                    )
            nc.sync.dma_start(out=col_d.ap(), in_=colbuf[:])

    nc.compile()
    return nc


def _get_compiled():
    global _compiled
    if _compiled is None:
        _compiled = _build()
    return _compiled


def _make_core_inputs(pred, target):
    """Per-core augmented, transposed fp16 operands."""
    ins = []
    for c in range(8):
        b, h = c // 2, c % 2
        pl = pred[b, h * N_LOC : (h + 1) * N_LOC]  # [N_LOC, 32]
        tg = target[b]  # [M, 32]
        pt = np.empty((K_AUG, N_LOC), dtype=np.float32)
        pt[:D] = -2.0 * pl.T
        pt[D] = np.sum(pl * pl, axis=1)
        pt[D + 1] = 1.0
        tt = np.empty((K_AUG, M), dtype=np.float32)
        tt[:D] = tg.T
        tt[D] = 1.0
        tt[D + 1] = np.sum(tg * tg, axis=1)
        ins.append(
            {
                "pt": np.ascontiguousarray(pt.astype(np.float16)),
                "tt": np.ascontiguousarray(tt.astype(np.float16)),
            }
        )
    return ins


def _finish(results):
    """Host tail: combine per-core partial minima into the scalar loss."""
    row_sum = 0.0
    col_sum = 0.0
    for b in range(B):
        col_d2 = None
        for h in range(2):
            r = results[2 * b + h]
            # rowcand[p, i, jj, q]: min over (jj, q) -> row n = 128*i + p
            rc = np.asarray(r["rowcand"], dtype=np.float32)
            rm = rc.reshape(128, NI, NJJ * 512).min(axis=2)
            row_sum += np.sum(np.sqrt(np.maximum(rm.astype(np.float64), 0.0)))
            cm = np.asarray(r["colmin"], dtype=np.float64).min(axis=0).reshape(M)
            col_d2 = cm if col_d2 is None else np.minimum(col_d2, cm)
        col_sum += np.sum(np.sqrt(np.maximum(col_d2, 0.0)))
    total = row_sum / (B * N) + col_sum / (B * M)
    return np.float32(total)


def kernel(pred, target, **run_kwargs):
    from concourse.bass_utils import run_bass_kernel_spmd

    pred = np.asarray(pred, dtype=np.float32)
    target = np.asarray(target, dtype=np.float32)
    nc = _get_compiled()
    ins = _make_core_inputs(pred, target)
    res = run_bass_kernel_spmd(nc, ins, list(range(8)), **run_kwargs)
    out = _finish(res.results)
    if run_kwargs:
        return out, res
    return out
